# revision 8
# baseline (speedup 1.0000x reference)
"""Trainium2 Bass kernel for nn_BiasAttention (B=8, N=2048, C=256, H=8).

Sharding: data-parallel over batch B across the 8 NeuronCores (one batch
element per core).  Weights and atten_bias are replicated to every core.

Per-core dataflow (everything kept "transposed" so the contraction dim is
always on SBUF partitions):
  xT   = x^T                              [C, N]     (PE transpose)
  qT,kT = wqkv rows 0..511 @ xT           [256, N]   (heads h at partitions 32h)
  v    = x @ wv^T                         [N, 256]   (natural, lhsT for attnv)
  sigT = sigmoid(atten_bias)^T            [N, N]     bf16, PE transpose + ACT
  per head-group hg (4 heads) and query block nb (512):
    S^T[m,n] = kT.T @ qT      4 heads concurrently via row-packed K=32 matmuls
    P = S^T * sigT            DVE tensor_tensor straight from PSUM -> bf16
    E = exp(P / sqrt(D))      ACT, in place
    outT[d,n] += v^T E        4 heads via col-packed M=32 matmuls
    Z[n]    += ones^T E       col-packed M=1 matmuls (softmax denominator)
  out = outT / Z              (reciprocal + partition-broadcast DMA)
  yT = wproj^T @ out          then + b_proj, PE transpose, DMA out
"""

import math

import numpy as np

B, N, C, H = 8, 2048, 256, 8
D = C // H  # 32
NCORES = 8
HG = 2  # head groups of 4
NB = N // 512  # 4 query blocks
MT = N // 128  # 16 key tiles

_cache = {}


def _build_module():
    import concourse.bacc as bacc
    import concourse.mybir as mybir
    import concourse.tile as tile
    from concourse.bass import ds, ts
    from concourse.masks import make_identity

    f32 = mybir.dt.float32
    bf16 = mybir.dt.bfloat16
    AF = mybir.ActivationFunctionType
    MUL = mybir.AluOpType.mult

    nc = bacc.Bacc("TRN2", target_bir_lowering=False, debug=False,
                   num_devices=NCORES)

    x_d = nc.dram_tensor("x", [N, C], f32, kind="ExternalInput")
    bias_d = nc.dram_tensor("atten_bias", [N, N], f32, kind="ExternalInput")
    wqkv_d = nc.dram_tensor("w_qkv", [3 * C, C], f32, kind="ExternalInput")
    wproj_d = nc.dram_tensor("w_proj", [C, C], f32, kind="ExternalInput")
    bproj_d = nc.dram_tensor("b_proj", [C], f32, kind="ExternalInput")
    y_d = nc.dram_tensor("y", [N, C], f32, kind="ExternalOutput")

    with tile.TileContext(nc) as tc:
        with (
            tc.tile_pool(name="const", bufs=1) as const,
            tc.tile_pool(name="big", bufs=1) as big,
        ):
            ident = const.tile([128, 128], f32)
            make_identity(nc, ident)
            ones_col = const.tile([128, 1], bf16)
            nc.vector.memset(ones_col, 1.0)
            bproj_sb = const.tile([128, 2], f32)
            nc.sync.dma_start(bproj_sb, bproj_d[:].rearrange("(j p) -> p j", p=128))

            wqkvT = const.tile([128, 2, 768], bf16)   # [c, cc, o]
            wprojT = const.tile([128, 2, 256], bf16)  # [c, cc, j]
            xT = big.tile([128, 2, N], bf16)          # [c, cc, n]
            qT = big.tile([128, HG, N], bf16)         # [32h+d, hg, n]
            kT = big.tile([128, HG, N], bf16)
            v_sb = big.tile([128, MT, 256], bf16)     # [m, mt, h*32+d]
            sigT = big.tile([128, MT, N], bf16)       # [m, mt, n]
            outT = big.tile([128, HG, N], f32)        # [32h+d, hg, n]
            z_sb = big.tile([8, N], f32)              # [h, n]
            rz_sb = big.tile([8, N], f32)

            # ---------------- P0: weights + x transpose + qkv ----------------
            with (
                tc.tile_pool(name="stage", bufs=3) as stage,
                tc.tile_pool(name="tpsum", bufs=2, space="PSUM") as tpsum,
                tc.tile_pool(name="qpsum", bufs=2, space="PSUM") as qpsum,
            ):
                # w_qkv^T
                for ot in range(6):
                    wst = stage.tile([128, 256], f32, tag="wst")
                    nc.sync.dma_start(wst, wqkv_d[ts(ot, 128), :])
                    pst = tpsum.tile([128, 256], f32, tag="tp")
                    for cc in range(2):
                        nc.tensor.transpose(pst[:, ts(cc, 128)], wst[:, ts(cc, 128)],
                                            ident)
                    for cc in range(2):
                        nc.scalar.copy(wqkvT[:, cc, ts(ot, 128)], pst[:, ts(cc, 128)])
                # w_proj^T
                for jt in range(2):
                    wst = stage.tile([128, 256], f32, tag="wst")
                    nc.sync.dma_start(wst, wproj_d[ts(jt, 128), :])
                    pst = tpsum.tile([128, 256], f32, tag="tp")
                    for cc in range(2):
                        nc.tensor.transpose(pst[:, ts(cc, 128)], wst[:, ts(cc, 128)],
                                            ident)
                    for cc in range(2):
                        nc.scalar.copy(wprojT[:, cc, ts(jt, 128)], pst[:, ts(cc, 128)])
                # x^T
                for nt in range(MT):
                    xst = stage.tile([128, 256], f32, tag="xst")
                    nc.sync.dma_start(xst, x_d[ts(nt, 128), :])
                    pst = tpsum.tile([128, 256], f32, tag="tp")
                    for cc in range(2):
                        nc.tensor.transpose(pst[:, ts(cc, 128)], xst[:, ts(cc, 128)],
                                            ident)
                    for cc in range(2):
                        nc.scalar.copy(xT[:, cc, ts(nt, 128)], pst[:, ts(cc, 128)])
                # qT, kT  (o tiles 0,1 -> q ; 2,3 -> k)
                for og in range(4):
                    dest = qT if og < 2 else kT
                    hg = og % 2
                    for nb in range(NB):
                        ps = qpsum.tile([128, 512], f32, tag="qp")
                        for cc in range(2):
                            nc.tensor.matmul(ps, wqkvT[:, cc, ts(og, 128)],
                                             xT[:, cc, ts(nb, 512)],
                                             start=(cc == 0), stop=(cc == 1))
                        nc.scalar.copy(dest[:, hg, ts(nb, 512)], ps)
                # v (natural layout)
                for mt in range(MT):
                    ps = qpsum.tile([128, 256], f32, tag="vp")
                    for cc in range(2):
                        nc.tensor.matmul(ps, xT[:, cc, ts(mt, 128)],
                                         wqkvT[:, cc, 512:768],
                                         start=(cc == 0), stop=(cc == 1))
                    nc.scalar.copy(v_sb[:, mt, :], ps)

            # ---------------- P1: sigmoid(atten_bias)^T ----------------------
            with (
                tc.tile_pool(name="bstage", bufs=2) as bstage,
                tc.tile_pool(name="bpsum", bufs=2, space="PSUM") as bpsum,
            ):
                for nt in range(MT):
                    bst = bstage.tile([128, N], f32, tag="bst")
                    nc.sync.dma_start(bst, bias_d[ts(nt, 128), :])
                    for mq in range(4):
                        pst = bpsum.tile([128, 512], f32, tag="bp")
                        for j in range(4):
                            nc.tensor.transpose(pst[:, ts(j, 128)],
                                                bst[:, ts(mq * 4 + j, 128)], ident)
                        nc.scalar.activation(
                            sigT[:, mq * 4:(mq + 1) * 4, ts(nt, 128)],
                            pst.rearrange("p (j f) -> p j f", j=4),
                            AF.Sigmoid)

            # ---------------- P2: attention ---------------------------------
            scale = float(D ** -0.5)
            with (
                tc.tile_pool(name="epool", bufs=4) as epool,
                tc.tile_pool(name="zstage", bufs=2) as zstage,
                tc.tile_pool(name="scps", bufs=1, space="PSUM") as scps,
                tc.tile_pool(name="outps", bufs=2, space="PSUM") as outps,
                tc.tile_pool(name="zps", bufs=2, space="PSUM") as zps,
            ):
                for hg in range(HG):
                    for nb in range(NB):
                        out_ps = outps.tile([128, 512], f32, tag="o")
                        z_ps = zps.tile([128, 512], f32, tag="z")
                        for mq in range(4):
                            e_t = epool.tile([128, 4, 4, 512], bf16, tag="e")
                            for j in range(4):
                                mt = mq * 4 + j
                                sc = scps.tile([128, 4, 512], f32, tag="s")
                                for h in range(4):
                                    nc.tensor.matmul(
                                        sc[:, h, :],
                                        kT[32 * h:32 * (h + 1), hg, ts(mt, 128)],
                                        qT[32 * h:32 * (h + 1), hg, ts(nb, 512)],
                                        start=True, stop=True,
                                        tile_position=(32 * h, 0))
                                nc.vector.tensor_tensor(
                                    e_t[:, j], sc,
                                    sigT[:, mt:mt + 1, ts(nb, 512)].to_broadcast(
                                        (128, 4, 512)),
                                    MUL)
                            nc.scalar.activation(e_t, e_t, AF.Exp, scale=scale)
                            for j in range(4):
                                mt = mq * 4 + j
                                first = mt == 0
                                last = mt == MT - 1
                                for h in range(4):
                                    nc.tensor.matmul(
                                        out_ps[32 * h:32 * (h + 1), :],
                                        v_sb[:, mt,
                                             (hg * 4 + h) * 32:(hg * 4 + h + 1) * 32],
                                        e_t[:, j, h],
                                        start=first, stop=last,
                                        tile_position=(0, 32 * h))
                                for h in range(4):
                                    nc.tensor.matmul(
                                        z_ps[32 * h:32 * h + 1, :],
                                        ones_col,
                                        e_t[:, j, h],
                                        start=first, stop=last,
                                        tile_position=(0, 32 * h))
                        nc.scalar.copy(outT[:, hg, ts(nb, 512)], out_ps)
                        z_st = zstage.tile([128, 512], f32, tag="zst")
                        nc.scalar.copy(z_st, z_ps)
                        nc.sync.dma_start(z_sb[4 * hg:4 * hg + 4, ts(nb, 512)],
                                          z_st[0:128:32, :])

            # ---------------- P3: normalize + proj + output ------------------
            with (
                tc.tile_pool(name="fpool", bufs=1) as fpool,
                tc.tile_pool(name="rzpool", bufs=2) as rzpool,
                tc.tile_pool(name="ystage", bufs=3) as ystage,
                tc.tile_pool(name="dpool", bufs=1, space="DRAM") as dpool,
                tc.tile_pool(name="pps", bufs=2, space="PSUM") as pps,
                tc.tile_pool(name="yps", bufs=3, space="PSUM") as yps,
            ):
                nc.vector.reciprocal(rz_sb, z_sb)
                rz_dram = dpool.tile([8, N], f32)
                nc.sync.dma_start(rz_dram, rz_sb)
                outTn = fpool.tile([128, HG, N], bf16)
                yT = fpool.tile([128, 2, N], f32)
                for hg in range(HG):
                    rz_bc = rzpool.tile([128, N], f32, tag="rz")
                    for h in range(4):
                        nc.sync.dma_start(
                            rz_bc[32 * h:32 * (h + 1), :],
                            rz_dram[4 * hg + h:4 * hg + h + 1, :].to_broadcast((32, N)))
                    nc.vector.tensor_tensor(outTn[:, hg], outT[:, hg], rz_bc, MUL)
                for jt in range(2):
                    for nb in range(NB):
                        ps = pps.tile([128, 512], f32, tag="pp")
                        for cc in range(2):
                            nc.tensor.matmul(ps, wprojT[:, cc, ts(jt, 128)],
                                             outTn[:, cc, ts(nb, 512)],
                                             start=(cc == 0), stop=(cc == 1))
                        nc.vector.tensor_scalar_add(yT[:, jt, ts(nb, 512)], ps,
                                                    bproj_sb[:, jt:jt + 1])
                for nt in range(MT):
                    yo = yps.tile([128, 256], f32, tag="yo")
                    for jt in range(2):
                        nc.tensor.transpose(yo[:, ts(jt, 128)],
                                            yT[:, jt, ts(nt, 128)], ident)
                    y_st = ystage.tile([128, 256], f32, tag="yst")
                    nc.scalar.copy(y_st, yo)
                    nc.sync.dma_start(y_d[ts(nt, 128), :], y_st)

    nc.compile()
    return nc


def _get_module():
    if "nc" not in _cache:
        _cache["nc"] = _build_module()
    return _cache["nc"]


def kernel(x, atten_bias, w_qkv, w_proj, b_proj, _results_out=None):
    from concourse.bass_utils import run_bass_kernel_spmd

    nc = _get_module()
    x = np.asarray(x, dtype=np.float32)
    atten_bias = np.ascontiguousarray(np.asarray(atten_bias, dtype=np.float32))
    w_qkv = np.ascontiguousarray(np.asarray(w_qkv, dtype=np.float32))
    w_proj = np.ascontiguousarray(np.asarray(w_proj, dtype=np.float32))
    b_proj = np.ascontiguousarray(np.asarray(b_proj, dtype=np.float32))

    in_maps = [
        {
            "x": np.ascontiguousarray(x[b]),
            "atten_bias": atten_bias,
            "w_qkv": w_qkv,
            "w_proj": w_proj,
            "b_proj": b_proj,
        }
        for b in range(B)
    ]
    res = run_bass_kernel_spmd(nc, in_maps, core_ids=list(range(NCORES)))
    if _results_out is not None:
        _results_out.append(res)
    out = np.stack([res.results[b]["y"] for b in range(B)], axis=0)
    return out.astype(np.float32)
